# revision 2
# baseline (speedup 1.0000x reference)
"""Causal attention (QKV proj + causal softmax + AV) on 8 TRN2 NeuronCores.

Sharding: Q rows sharded over cores (strided 128-row blocks for causal load
balance); K/V projections computed redundantly on every core (cheaper than
on-chip collectives at this size). Flash-style online softmax over 512-key
chunks so K/V tiles never round-trip through DRAM.

Core c owns 128-row blocks B in {c, 8+c, 16+c, 24+c}. All cores run one
identical static program; the causal mask comes in as per-core threshold
data (wthr), so a single NEFF serves all 8 cores.
"""

import numpy as np
import ml_dtypes
from contextlib import ExitStack

import concourse.bass as bass
import concourse.tile as tile
from concourse import bacc, mybir
from concourse.bass_utils import run_bass_kernel_spmd
from concourse.masks import make_identity

P = 128
SEQ = 4096
D = 1024
N_CORES = 8
RPC = SEQ // N_CORES          # 512 rows of Q per core
D_TILES = D // P              # 8
KCHUNK = 512                  # key chunk width
SEQ_CHUNKS = SEQ // KCHUNK    # 8
N_QTILES = RPC // P           # 4 local Q tiles per core
TILE_CHUNKS = [2, 4, 6, 8]    # key chunks per local Q tile (uniform superstructure)
SM_SCALE = 1.0 / 32.0         # 1/sqrt(D)
NEG_BIG = -1.0e9

BF16 = mybir.dt.bfloat16
F32 = mybir.dt.float32

_CACHE = {}


def _build_nc():
    nc = bacc.Bacc("TRN2", target_bir_lowering=False, debug=False,
                   num_devices=N_CORES)

    xT = nc.dram_tensor("xT", [D, SEQ], BF16, kind="ExternalInput").ap()
    xTq = nc.dram_tensor("xTq", [D, RPC], BF16, kind="ExternalInput").ap()
    wqT = nc.dram_tensor("wqT", [D, D], BF16, kind="ExternalInput").ap()
    wkT = nc.dram_tensor("wkT", [D, D], BF16, kind="ExternalInput").ap()
    wvT = nc.dram_tensor("wvT", [D, D], BF16, kind="ExternalInput").ap()
    wthr = nc.dram_tensor("wthr", [P, N_QTILES * SEQ_CHUNKS], F32,
                          kind="ExternalInput").ap()
    out = nc.dram_tensor("out", [RPC, D], F32, kind="ExternalOutput").ap()

    xT_t = xT.rearrange("(o p) f -> p o f", p=P)       # [128, 8, 4096]
    xTq_t = xTq.rearrange("(o p) f -> p o f", p=P)     # [128, 8, 512]
    wq_t = wqT.rearrange("(o p) f -> p o f", p=P)      # [128, 8, 1024]
    wk_t = wkT.rearrange("(o p) f -> p o f", p=P)
    wv_t = wvT.rearrange("(o p) f -> p o f", p=P)
    out_t = out.rearrange("(t p) f -> p t f", p=P)     # [128, 4, 1024]

    with tile.TileContext(nc) as tc, ExitStack() as ctx:
        _kernel(ctx, tc, xT_t, xTq_t, wq_t, wk_t, wv_t, wthr, out_t)
    nc.compile()
    return nc


def _kernel(ctx, tc, xT_t, xTq_t, wq_t, wk_t, wv_t, wthr, out_t):
    nc = tc.nc
    AX = mybir.AxisListType
    OP = mybir.AluOpType
    ACT = mybir.ActivationFunctionType

    consts = ctx.enter_context(tc.tile_pool(name="consts", bufs=1))
    wpool = ctx.enter_context(tc.tile_pool(name="w", bufs=1))
    qt_pool = ctx.enter_context(tc.tile_pool(name="qt", bufs=1))
    xs_pool = ctx.enter_context(tc.tile_pool(name="xs", bufs=3))
    kt_pool = ctx.enter_context(tc.tile_pool(name="kt", bufs=3))
    vv_pool = ctx.enter_context(tc.tile_pool(name="vv", bufs=3))
    s_pool = ctx.enter_context(tc.tile_pool(name="s", bufs=3))
    ge_pool = ctx.enter_context(tc.tile_pool(name="ge", bufs=3))
    p_pool = ctx.enter_context(tc.tile_pool(name="p", bufs=3))
    pt_pool = ctx.enter_context(tc.tile_pool(name="pt", bufs=3))
    o_pool = ctx.enter_context(tc.tile_pool(name="obig", bufs=1))
    osb_pool = ctx.enter_context(tc.tile_pool(name="osb", bufs=2))
    stat_pool = ctx.enter_context(tc.tile_pool(name="stat", bufs=48))

    proj_ps = ctx.enter_context(tc.tile_pool(name="proj_ps", bufs=2, space="PSUM"))
    s_ps = ctx.enter_context(tc.tile_pool(name="s_ps", bufs=2, space="PSUM"))
    t_ps = ctx.enter_context(tc.tile_pool(name="t_ps", bufs=2, space="PSUM"))
    o_ps = ctx.enter_context(tc.tile_pool(name="o_ps", bufs=2, space="PSUM"))

    # ---- constants -------------------------------------------------------
    ident = consts.tile([P, P], BF16)
    make_identity(nc, ident)

    iota_i = consts.tile([P, KCHUNK], mybir.dt.int32)
    nc.gpsimd.iota(iota_i, pattern=[[1, KCHUNK]], base=0, channel_multiplier=0)
    iota_f = consts.tile([P, KCHUNK], F32)
    nc.vector.tensor_copy(iota_f, iota_i)

    wthr_sb = consts.tile([P, N_QTILES * SEQ_CHUNKS], F32)
    nc.sync.dma_start(out=wthr_sb[:], in_=wthr)

    # ---- weights ---------------------------------------------------------
    wq_sb = wpool.tile([P, D_TILES, D], BF16, tag="wq")
    wk_sb = wpool.tile([P, D_TILES, D], BF16, tag="wk")
    wv_sb = wpool.tile([P, D_TILES, D], BF16, tag="wv")
    nc.sync.dma_start(out=wq_sb[:], in_=wq_t)
    nc.sync.dma_start(out=wk_sb[:], in_=wk_t)
    nc.sync.dma_start(out=wv_sb[:], in_=wv_t)

    # ---- Q^T projection: qt[do_p, do_o, local_row] -----------------------
    xq_sb = xs_pool.tile([P, D_TILES, RPC], BF16, tag="xs")
    nc.sync.dma_start(out=xq_sb[:], in_=xTq_t)
    qt_sb = qt_pool.tile([P, D_TILES, RPC], BF16)
    for do in range(D_TILES):
        ps = proj_ps.tile([P, RPC], F32)
        for di in range(D_TILES):
            nc.tensor.matmul(ps, wq_sb[:, di, do * P:(do + 1) * P],
                             xq_sb[:, di, :],
                             start=(di == 0), stop=(di == D_TILES - 1))
        nc.vector.tensor_copy(qt_sb[:, do, :], ps)

    # running stats + output accumulator per local Q tile
    o_all = o_pool.tile([P, N_QTILES, D], F32)
    m_run = [None] * N_QTILES
    l_run = [None] * N_QTILES

    # ---- main loop over key chunks --------------------------------------
    for j in range(SEQ_CHUNKS):
        # K^T chunk: kt[do_p, do_o, key] and V chunk: v[key_p, key_o, dv]
        xs = xs_pool.tile([P, D_TILES, KCHUNK], BF16, tag="xs")
        nc.sync.dma_start(out=xs[:], in_=xT_t[:, :, j * KCHUNK:(j + 1) * KCHUNK])

        ktj = kt_pool.tile([P, D_TILES, KCHUNK], BF16)
        for do in range(D_TILES):
            ps = proj_ps.tile([P, KCHUNK], F32)
            for di in range(D_TILES):
                nc.tensor.matmul(ps, wk_sb[:, di, do * P:(do + 1) * P],
                                 xs[:, di, :],
                                 start=(di == 0), stop=(di == D_TILES - 1))
            nc.vector.tensor_copy(ktj[:, do, :], ps)

        vj = vv_pool.tile([P, 4, D], BF16)
        for ks in range(4):
            for h in range(2):
                ps = proj_ps.tile([P, KCHUNK], F32)
                for di in range(D_TILES):
                    nc.tensor.matmul(ps, xs[:, di, ks * P:(ks + 1) * P],
                                     wv_sb[:, di, h * 512:(h + 1) * 512],
                                     start=(di == 0), stop=(di == D_TILES - 1))
                nc.vector.tensor_copy(vj[:, ks, h * 512:(h + 1) * 512], ps)

        # attention for every local Q tile that touches this key chunk
        for t in range(N_QTILES):
            if j >= TILE_CHUNKS[t]:
                continue
            first = (j == 0)

            sps = s_ps.tile([P, KCHUNK], F32)
            for do in range(D_TILES):
                nc.tensor.matmul(sps, qt_sb[:, do, t * P:(t + 1) * P],
                                 ktj[:, do, :],
                                 start=(do == 0), stop=(do == D_TILES - 1))

            # causal mask: masked iff iota >= wthr[:, t*8+j]
            col = t * SEQ_CHUNKS + j
            ge = ge_pool.tile([P, KCHUNK], F32)
            nc.vector.tensor_scalar(ge, iota_f, wthr_sb[:, col:col + 1], None,
                                    op0=OP.is_ge)
            nc.vector.tensor_scalar_mul(ge, ge, NEG_BIG)
            s_sb = s_pool.tile([P, KCHUNK], F32)
            nc.vector.tensor_tensor(s_sb, sps, ge, OP.add)

            # online softmax update
            rmax = stat_pool.tile([P, 1], F32, tag="stat")
            nc.vector.reduce_max(rmax, s_sb, axis=AX.X)
            if first:
                m_new = rmax
            else:
                m_new = stat_pool.tile([P, 1], F32, tag="stat")
                nc.vector.tensor_tensor(m_new, m_run[t], rmax, OP.max)
            nm = stat_pool.tile([P, 1], F32, tag="stat")
            nc.vector.tensor_scalar_mul(nm, m_new, -SM_SCALE)

            p_sb = p_pool.tile([P, KCHUNK], BF16)
            rsum = stat_pool.tile([P, 1], F32, tag="stat")
            nc.scalar.activation(p_sb, s_sb, ACT.Exp, bias=nm, scale=SM_SCALE,
                                 accum_out=rsum)

            if first:
                l_new = rsum
            else:
                alpha = stat_pool.tile([P, 1], F32, tag="stat")
                nc.scalar.activation(alpha, m_run[t], ACT.Exp, bias=nm,
                                     scale=SM_SCALE)
                l_new = stat_pool.tile([P, 1], F32, tag="stat")
                nc.vector.scalar_tensor_tensor(l_new, l_run[t], alpha, rsum,
                                               op0=OP.mult, op1=OP.add)
            m_run[t], l_run[t] = m_new, l_new

            # P^T tiles via PE transpose
            ptj = pt_pool.tile([P, 4, P], BF16)
            for ks in range(4):
                tps = t_ps.tile([P, P], BF16)
                nc.tensor.transpose(tps, p_sb[:, ks * P:(ks + 1) * P], ident)
                nc.vector.tensor_copy(ptj[:, ks, :], tps)

            # O update: O = O*alpha + P^T.T @ V
            for h in range(2):
                ops = o_ps.tile([P, KCHUNK], F32)
                for ks in range(4):
                    nc.tensor.matmul(ops, ptj[:, ks, :],
                                     vj[:, ks, h * 512:(h + 1) * 512],
                                     start=(ks == 0), stop=(ks == 3))
                o_slice = o_all[:, t, h * 512:(h + 1) * 512]
                if first:
                    nc.vector.tensor_copy(o_slice, ops)
                else:
                    nc.vector.scalar_tensor_tensor(o_slice, o_slice, alpha, ops,
                                                   op0=OP.mult, op1=OP.add)

    # ---- finalize: out = O / l ------------------------------------------
    for t in range(N_QTILES):
        recip = stat_pool.tile([P, 1], F32, tag="stat")
        nc.vector.reciprocal(recip, l_run[t])
        osb = osb_pool.tile([P, D], F32)
        nc.vector.tensor_scalar_mul(osb, o_all[:, t, :], recip)
        nc.sync.dma_start(out=out_t[:, t, :], in_=osb)


def _get_nc():
    if "nc" not in _CACHE:
        _CACHE["nc"] = _build_nc()
    return _CACHE["nc"]


def kernel(x, w_q, w_k, w_v):
    nc = _get_nc()
    bf = ml_dtypes.bfloat16

    xT = np.ascontiguousarray(x.T).astype(bf)              # [D, SEQ]
    wqT = np.ascontiguousarray(w_q.T).astype(bf)           # [d_in, d_out]
    wkT = np.ascontiguousarray(w_k.T).astype(bf)
    wvT = np.ascontiguousarray(w_v.T).astype(bf)

    in_maps = []
    for c in range(N_CORES):
        blocks = [8 * t + c for t in range(N_QTILES)]
        cols = np.concatenate([np.arange(b * P, (b + 1) * P) for b in blocks])
        xTq = np.ascontiguousarray(xT[:, cols])

        wthr = np.zeros((P, N_QTILES * SEQ_CHUNKS), np.float32)
        r = np.arange(P)
        for t, B in enumerate(blocks):
            for j in range(TILE_CHUNKS[t]):
                w = np.clip(128 * B + r + 1 - KCHUNK * j, 0, KCHUNK)
                wthr[:, t * SEQ_CHUNKS + j] = w

        in_maps.append({"xT": xT, "xTq": xTq, "wqT": wqT, "wkT": wkT,
                        "wvT": wvT, "wthr": wthr})

    res = run_bass_kernel_spmd(nc, in_maps, list(range(N_CORES)))

    full = np.empty((SEQ, D), np.float32)
    for c in range(N_CORES):
        oc = res.results[c]["out"]
        for t in range(N_QTILES):
            B = 8 * t + c
            full[B * P:(B + 1) * P, :] = oc[t * P:(t + 1) * P, :]
    return full


# revision 3
# speedup vs baseline: 1.0372x; 1.0372x over previous
"""Causal attention on 8 TRN2 NeuronCores — two-phase version.

Phase 1 (NEFF-1): Q/K/V projections. K/V sharded over seq across cores;
Q^T computed for the core's own (strided) row blocks.
Host: stack the per-core K^T / V shards (pure data movement).
Phase 2 (NEFF-2): flash-style causal attention, Q rows sharded over cores
(strided 128-row blocks), K^T/V streamed chunk-wise from DRAM.

All DRAM tensors use SBUF-mirroring layouts (partition dim first) so every
DMA is contiguous per partition.
"""

import numpy as np
import ml_dtypes
from contextlib import ExitStack

import concourse.bass as bass
import concourse.tile as tile
from concourse import bacc, mybir
from concourse.bass_utils import run_bass_kernel_spmd
from concourse.masks import make_identity

P = 128
SEQ = 4096
D = 1024
N_CORES = 8
RPC = SEQ // N_CORES          # 512
D_TILES = D // P              # 8
KCHUNK = 512
SEQ_CHUNKS = SEQ // KCHUNK    # 8
N_QTILES = RPC // P           # 4
TILE_CHUNKS = [2, 4, 6, 8]
N_PAIRS = sum(TILE_CHUNKS)    # 20
SM_SCALE = 1.0 / 32.0
NEG_BIG = -1.0e9

BF16 = mybir.dt.bfloat16
F32 = mybir.dt.float32

_CACHE = {}


# ---------------------------------------------------------------- NEFF 1
def _build_nc1():
    nc = bacc.Bacc("TRN2", target_bir_lowering=False, debug=False,
                   num_devices=N_CORES)
    # pre-permuted layouts: partition dim first, contiguous per partition
    xc = nc.dram_tensor("xc", [P, D_TILES, KCHUNK], BF16,
                        kind="ExternalInput").ap()
    xq = nc.dram_tensor("xq", [P, D_TILES, RPC], BF16,
                        kind="ExternalInput").ap()
    wk = nc.dram_tensor("wk", [D_TILES, P, D_TILES, P], BF16,
                        kind="ExternalInput").ap()
    wq = nc.dram_tensor("wq", [D_TILES, P, D_TILES, P], BF16,
                        kind="ExternalInput").ap()
    wv = nc.dram_tensor("wv", [2, P, D_TILES, KCHUNK], BF16,
                        kind="ExternalInput").ap()
    kt_o = nc.dram_tensor("kt", [P, D_TILES, KCHUNK], BF16,
                          kind="ExternalOutput").ap()
    v_o = nc.dram_tensor("v", [P, 4, D], BF16, kind="ExternalOutput").ap()
    qt_o = nc.dram_tensor("qt", [P, D_TILES, RPC], BF16,
                          kind="ExternalOutput").ap()

    with tile.TileContext(nc) as tc, ExitStack() as ctx:
        wpool = ctx.enter_context(tc.tile_pool(name="w", bufs=1))
        xpool = ctx.enter_context(tc.tile_pool(name="x", bufs=1))
        opool = ctx.enter_context(tc.tile_pool(name="o", bufs=6))
        ps = ctx.enter_context(tc.tile_pool(name="ps", bufs=4, space="PSUM"))

        xs = xpool.tile([P, D_TILES, KCHUNK], BF16, tag="xs")
        for di in range(D_TILES):
            nc.sync.dma_start(out=xs[:, di, :], in_=xc[:, di, :])

        # weight SBUF layouts mirror the chunked DRAM layouts:
        # wk_sb/wq_sb: [di_p, do_chunk, di_o, do_i]; wv_sb: [di_p, half, di_o, do_i]
        wk_sb = wpool.tile([P, D_TILES, D_TILES, P], BF16, tag="wk")
        wq_sb = wpool.tile([P, D_TILES, D_TILES, P], BF16, tag="wq")
        wv_sb = wpool.tile([P, 2, D_TILES, KCHUNK], BF16, tag="wv")
        for do in range(D_TILES):
            nc.sync.dma_start(out=wk_sb[:, do], in_=wk[do])
        xq_sb = xpool.tile([P, D_TILES, RPC], BF16, tag="xq")
        nc.sync.dma_start(out=xq_sb[:], in_=xq)
        for do in range(D_TILES):
            nc.sync.dma_start(out=wq_sb[:, do], in_=wq[do])
        for h in range(2):
            nc.sync.dma_start(out=wv_sb[:, h], in_=wv[h])

        for do in range(D_TILES):
            p = ps.tile([P, KCHUNK], F32)
            for di in range(D_TILES):
                nc.tensor.matmul(p, wk_sb[:, do, di, :],
                                 xs[:, di, :],
                                 start=(di == 0), stop=(di == D_TILES - 1))
            o = opool.tile([P, KCHUNK], BF16, tag="o")
            nc.vector.tensor_copy(o, p)
            nc.sync.dma_start(out=kt_o[:, do, :], in_=o)

        for do in range(D_TILES):
            p = ps.tile([P, RPC], F32)
            for di in range(D_TILES):
                nc.tensor.matmul(p, wq_sb[:, do, di, :],
                                 xq_sb[:, di, :],
                                 start=(di == 0), stop=(di == D_TILES - 1))
            o = opool.tile([P, RPC], BF16, tag="o")
            nc.vector.tensor_copy(o, p)
            nc.sync.dma_start(out=qt_o[:, do, :], in_=o)

        for ks in range(4):
            for h in range(2):
                p = ps.tile([P, KCHUNK], F32)
                for di in range(D_TILES):
                    nc.tensor.matmul(p, xs[:, di, ks * P:(ks + 1) * P],
                                     wv_sb[:, h, di, :],
                                     start=(di == 0), stop=(di == D_TILES - 1))
                o = opool.tile([P, KCHUNK], BF16, tag="o")
                nc.vector.tensor_copy(o, p)
                nc.sync.dma_start(out=v_o[:, ks, h * 512:(h + 1) * 512], in_=o)
    nc.compile()
    return nc


# ---------------------------------------------------------------- NEFF 2
def _build_nc2():
    nc = bacc.Bacc("TRN2", target_bir_lowering=False, debug=False,
                   num_devices=N_CORES)
    ktf = nc.dram_tensor("ktf", [SEQ_CHUNKS, P, D_TILES, KCHUNK], BF16,
                         kind="ExternalInput").ap()
    vf = nc.dram_tensor("vf", [SEQ_CHUNKS, P, 4, D], BF16,
                        kind="ExternalInput").ap()
    qt = nc.dram_tensor("qt", [P, D_TILES, RPC], BF16,
                        kind="ExternalInput").ap()
    wthr = nc.dram_tensor("wthr", [P, N_QTILES * SEQ_CHUNKS], F32,
                          kind="ExternalInput").ap()
    out = nc.dram_tensor("out", [RPC, D], F32, kind="ExternalOutput").ap()
    out_t = out.rearrange("(t p) f -> p t f", p=P)

    with tile.TileContext(nc) as tc, ExitStack() as ctx:
        _attention(ctx, tc, ktf, vf, qt, wthr, out_t)
    nc.compile()
    return nc


def _attention(ctx, tc, ktf, vf, qt_in, wthr, out_t):
    nc = tc.nc
    AX = mybir.AxisListType
    OP = mybir.AluOpType
    ACT = mybir.ActivationFunctionType

    consts = ctx.enter_context(tc.tile_pool(name="consts", bufs=1))
    qt_pool = ctx.enter_context(tc.tile_pool(name="qt", bufs=1))
    kt_pool = ctx.enter_context(tc.tile_pool(name="kt", bufs=3))
    vv_pool = ctx.enter_context(tc.tile_pool(name="vv", bufs=3))
    p_pool = ctx.enter_context(tc.tile_pool(name="p", bufs=3))
    pt_pool = ctx.enter_context(tc.tile_pool(name="pt", bufs=3))
    o_pool = ctx.enter_context(tc.tile_pool(name="obig", bufs=1))
    osb_pool = ctx.enter_context(tc.tile_pool(name="osb", bufs=2))
    stat_pool = ctx.enter_context(tc.tile_pool(name="stat", bufs=48))

    s_ps = ctx.enter_context(tc.tile_pool(name="s_ps", bufs=2, space="PSUM"))
    t_ps = ctx.enter_context(tc.tile_pool(name="t_ps", bufs=2, space="PSUM"))
    o_ps = ctx.enter_context(tc.tile_pool(name="o_ps", bufs=2, space="PSUM"))

    qt_sb = qt_pool.tile([P, D_TILES, RPC], BF16)
    nc.sync.dma_start(out=qt_sb[:], in_=qt_in)

    ident = consts.tile([P, P], BF16)
    make_identity(nc, ident)
    iota_i = consts.tile([P, KCHUNK], mybir.dt.int32)
    nc.gpsimd.iota(iota_i, pattern=[[1, KCHUNK]], base=0, channel_multiplier=0)
    iota_f = consts.tile([P, KCHUNK], F32)
    nc.vector.tensor_copy(iota_f, iota_i)
    wthr_sb = consts.tile([P, N_QTILES * SEQ_CHUNKS], F32)
    nc.sync.dma_start(out=wthr_sb[:], in_=wthr)

    negbig = consts.tile([P, KCHUNK], F32)
    nc.gpsimd.memset(negbig, NEG_BIG)
    masks = consts.tile([P, N_PAIRS, KCHUNK], F32)
    pair_idx = {}
    pi = 0
    # j-major so the j=0 masks are ready first; built on idle GPSIMD
    for j in range(SEQ_CHUNKS):
        for t in range(N_QTILES):
            if j >= TILE_CHUNKS[t]:
                continue
            col = t * SEQ_CHUNKS + j
            nc.vector.scalar_tensor_tensor(
                masks[:, pi, :], iota_f, wthr_sb[:, col:col + 1], negbig,
                op0=OP.is_ge, op1=OP.mult)
            pair_idx[(t, j)] = pi
            pi += 1

    o_all = o_pool.tile([P, N_QTILES, D], F32)
    m_run = [None] * N_QTILES
    l_run = [None] * N_QTILES

    for j in range(SEQ_CHUNKS):
        ktj = kt_pool.tile([P, D_TILES, KCHUNK], BF16, tag="kt")
        nc.sync.dma_start(out=ktj[:], in_=ktf[j])
        vj = vv_pool.tile([P, 4, D], BF16)
        nc.sync.dma_start(out=vj[:], in_=vf[j])

        for t in range(N_QTILES):
            if j >= TILE_CHUNKS[t]:
                continue
            first = (j == 0)

            sps = s_ps.tile([P, KCHUNK], F32)
            for do in range(D_TILES):
                nc.tensor.matmul(sps, qt_sb[:, do, t * P:(t + 1) * P],
                                 ktj[:, do, :],
                                 start=(do == 0), stop=(do == D_TILES - 1))

            nc.vector.tensor_tensor(sps, sps,
                                    masks[:, pair_idx[(t, j)], :], OP.add)
            rmax_s = stat_pool.tile([P, 1], F32, tag="stat")
            nc.vector.reduce_max(rmax_s, sps, axis=AX.X)

            # nm = -running_max of scaled scores (exp bias)
            nm = stat_pool.tile([P, 1], F32, tag="stat")
            if first:
                nc.vector.tensor_scalar_mul(nm, rmax_s, -SM_SCALE)
            else:
                nc.vector.scalar_tensor_tensor(nm, rmax_s, -SM_SCALE, m_run[t],
                                               op0=OP.mult, op1=OP.min)

            p_sb = p_pool.tile([P, KCHUNK], BF16)
            rsum = stat_pool.tile([P, 1], F32, tag="stat")
            nc.scalar.activation(p_sb, sps, ACT.Exp, bias=nm, scale=SM_SCALE,
                                 accum_out=rsum)

            if first:
                l_new = rsum
            else:
                # alpha = exp(nm_new - nm_old)
                alpha = stat_pool.tile([P, 1], F32, tag="stat")
                nc.scalar.activation(alpha, m_run[t], ACT.Exp, bias=nm,
                                     scale=-1.0)
                l_new = stat_pool.tile([P, 1], F32, tag="stat")
                nc.vector.scalar_tensor_tensor(l_new, l_run[t], alpha, rsum,
                                               op0=OP.mult, op1=OP.add)
            m_run[t], l_run[t] = nm, l_new

            tps = t_ps.tile([P, KCHUNK], BF16)
            for ks in range(4):
                nc.tensor.transpose(tps[:, ks * P:(ks + 1) * P],
                                    p_sb[:, ks * P:(ks + 1) * P], ident)
            ptj = pt_pool.tile([P, KCHUNK], BF16)
            nc.vector.tensor_copy(ptj, tps)

            ops = o_ps.tile([P, D], F32)
            for h in range(2):
                for ks in range(4):
                    nc.tensor.matmul(ops[:, h * 512:(h + 1) * 512],
                                     ptj[:, ks * P:(ks + 1) * P],
                                     vj[:, ks, h * 512:(h + 1) * 512],
                                     start=(ks == 0), stop=(ks == 3))
            o_slice = o_all[:, t, :]
            if first:
                nc.vector.tensor_copy(o_slice, ops)
            else:
                nc.vector.scalar_tensor_tensor(o_slice, o_slice, alpha, ops,
                                               op0=OP.mult, op1=OP.add)

    for t in range(N_QTILES):
        recip = stat_pool.tile([P, 1], F32, tag="stat")
        nc.vector.reciprocal(recip, l_run[t])
        osb = osb_pool.tile([P, D], F32)
        nc.vector.tensor_scalar_mul(osb, o_all[:, t, :], recip)
        nc.sync.dma_start(out=out_t[:, t, :], in_=osb)


def _get_ncs():
    if "nc1" not in _CACHE:
        _CACHE["nc1"] = _build_nc1()
        _CACHE["nc2"] = _build_nc2()
    return _CACHE["nc1"], _CACHE["nc2"]


def _qcols(c):
    blocks = [8 * t + c for t in range(N_QTILES)]
    return blocks, np.concatenate(
        [np.arange(b * P, (b + 1) * P) for b in blocks])


def _perm_x(xT_slice):
    """[D, W] -> [128, 8, W] with di_inner on partitions."""
    W = xT_slice.shape[1]
    return np.ascontiguousarray(
        xT_slice.reshape(D_TILES, P, W).transpose(1, 0, 2))


def _perm_w_chunks(wT):
    """[d_in, d_out] -> [8, 128, 8, 128]: [do_chunk, di_p, di_o, do_i]."""
    return np.ascontiguousarray(
        wT.reshape(D_TILES, P, D_TILES, P).transpose(2, 1, 0, 3))


def _perm_w_halves(wT):
    """[d_in, d_out] -> [2, 128, 8, 512]: [half, di_p, di_o, do_i]."""
    return np.ascontiguousarray(
        wT.reshape(D_TILES, P, 2, KCHUNK).transpose(2, 1, 0, 3))


def _phase1_inmaps(xT, wqT, wkT, wvT):
    wk_p = _perm_w_chunks(wkT)
    wq_p = _perm_w_chunks(wqT)
    wv_p = _perm_w_halves(wvT)
    maps = []
    for c in range(N_CORES):
        _, cols = _qcols(c)
        maps.append({
            "xc": _perm_x(xT[:, c * KCHUNK:(c + 1) * KCHUNK]),
            "xq": _perm_x(xT[:, cols]),
            "wq": wq_p, "wk": wk_p, "wv": wv_p})
    return maps


def _phase2_inmaps(ktf, vf, qts):
    maps = []
    r = np.arange(P)
    for c in range(N_CORES):
        blocks, _ = _qcols(c)
        wthr = np.zeros((P, N_QTILES * SEQ_CHUNKS), np.float32)
        for t, B in enumerate(blocks):
            for j in range(TILE_CHUNKS[t]):
                wthr[:, t * SEQ_CHUNKS + j] = np.clip(
                    128 * B + r + 1 - KCHUNK * j, 0, KCHUNK)
        maps.append({"ktf": ktf, "vf": vf, "qt": qts[c], "wthr": wthr})
    return maps


def kernel(x, w_q, w_k, w_v):
    nc1, nc2 = _get_ncs()
    bf = ml_dtypes.bfloat16
    xT = np.ascontiguousarray(x.T).astype(bf)
    wqT = np.ascontiguousarray(w_q.T).astype(bf)
    wkT = np.ascontiguousarray(w_k.T).astype(bf)
    wvT = np.ascontiguousarray(w_v.T).astype(bf)

    res1 = run_bass_kernel_spmd(nc1, _phase1_inmaps(xT, wqT, wkT, wvT),
                                list(range(N_CORES)))
    ktf = np.stack([res1.results[c]["kt"] for c in range(N_CORES)])
    vf = np.stack([res1.results[c]["v"] for c in range(N_CORES)])
    qts = [res1.results[c]["qt"] for c in range(N_CORES)]

    res2 = run_bass_kernel_spmd(nc2, _phase2_inmaps(ktf, vf, qts),
                                list(range(N_CORES)))

    full = np.empty((SEQ, D), np.float32)
    for c in range(N_CORES):
        oc = res2.results[c]["out"]
        blocks, _ = _qcols(c)
        for t, B in enumerate(blocks):
            full[B * P:(B + 1) * P, :] = oc[t * P:(t + 1) * P, :]
    return full


# revision 7
# speedup vs baseline: 1.0588x; 1.0208x over previous
"""Causal attention on 8 TRN2 NeuronCores — two-phase version.

Phase 1 (NEFF-1): Q/K/V projections. K/V sharded over seq across cores;
Q^T computed for the core's own (strided) row blocks.
Host: stack the per-core K^T / V shards (pure data movement).
Phase 2 (NEFF-2): flash-style causal attention, Q rows sharded over cores
(strided 128-row blocks), K^T/V streamed chunk-wise from DRAM.

All DRAM tensors use SBUF-mirroring layouts (partition dim first) so every
DMA is contiguous per partition.
"""

import numpy as np
import ml_dtypes
from contextlib import ExitStack

import concourse.bass as bass
import concourse.tile as tile
from concourse import bacc, mybir
from concourse.bass_utils import run_bass_kernel_spmd
from concourse.masks import make_identity

P = 128
SEQ = 4096
D = 1024
N_CORES = 8
RPC = SEQ // N_CORES          # 512
D_TILES = D // P              # 8
KCHUNK = 512
SEQ_CHUNKS = SEQ // KCHUNK    # 8
N_QTILES = RPC // P           # 4
TILE_CHUNKS = [2, 4, 6, 8]
N_PAIRS = sum(TILE_CHUNKS)    # 20
SM_SCALE = 1.0 / 32.0
NEG_BIG = -1.0e9

BF16 = mybir.dt.bfloat16
F32 = mybir.dt.float32

_CACHE = {}


# ---------------------------------------------------------------- NEFF 1
def _build_nc1():
    nc = bacc.Bacc("TRN2", target_bir_lowering=False, debug=False,
                   num_devices=N_CORES)
    # pre-permuted layouts: partition dim first, contiguous per partition
    xc = nc.dram_tensor("xc", [P, D_TILES, KCHUNK], BF16,
                        kind="ExternalInput").ap()
    xq = nc.dram_tensor("xq", [P, D_TILES, RPC], BF16,
                        kind="ExternalInput").ap()
    wk = nc.dram_tensor("wk", [D_TILES, P, D_TILES, P], BF16,
                        kind="ExternalInput").ap()
    wq = nc.dram_tensor("wq", [D_TILES, P, D_TILES, P], BF16,
                        kind="ExternalInput").ap()
    wv = nc.dram_tensor("wv", [2, P, D_TILES, KCHUNK], BF16,
                        kind="ExternalInput").ap()
    kt_o = nc.dram_tensor("kt", [P, D_TILES, KCHUNK], BF16,
                          kind="ExternalOutput").ap()
    v_o = nc.dram_tensor("v", [P, 4, D], BF16, kind="ExternalOutput").ap()
    qt_o = nc.dram_tensor("qt", [P, D_TILES, RPC], BF16,
                          kind="ExternalOutput").ap()

    with tile.TileContext(nc) as tc, ExitStack() as ctx:
        wpool = ctx.enter_context(tc.tile_pool(name="w", bufs=1))
        xpool = ctx.enter_context(tc.tile_pool(name="x", bufs=1))
        opool = ctx.enter_context(tc.tile_pool(name="o", bufs=6))
        ps = ctx.enter_context(tc.tile_pool(name="ps", bufs=4, space="PSUM"))

        xs = xpool.tile([P, D_TILES, KCHUNK], BF16, tag="xs")
        for di in range(D_TILES):
            nc.sync.dma_start(out=xs[:, di, :], in_=xc[:, di, :])

        # weight SBUF layouts mirror the chunked DRAM layouts:
        # wk_sb/wq_sb: [di_p, do_chunk, di_o, do_i]; wv_sb: [di_p, half, di_o, do_i]
        wk_sb = wpool.tile([P, D_TILES, D_TILES, P], BF16, tag="wk")
        wq_sb = wpool.tile([P, D_TILES, D_TILES, P], BF16, tag="wq")
        wv_sb = wpool.tile([P, 2, D_TILES, KCHUNK], BF16, tag="wv")
        for do in range(D_TILES):
            nc.sync.dma_start(out=wk_sb[:, do], in_=wk[do])
        for h in range(2):
            nc.sync.dma_start(out=wv_sb[:, h], in_=wv[h])
        xq_sb = xpool.tile([P, D_TILES, RPC], BF16, tag="xq")
        nc.sync.dma_start(out=xq_sb[:], in_=xq)
        for do in range(D_TILES):
            nc.sync.dma_start(out=wq_sb[:, do], in_=wq[do])

        for do in range(D_TILES):
            p = ps.tile([P, KCHUNK], F32)
            for di in range(D_TILES):
                nc.tensor.matmul(p, wk_sb[:, do, di, :],
                                 xs[:, di, :],
                                 start=(di == 0), stop=(di == D_TILES - 1))
            o = opool.tile([P, KCHUNK], BF16, tag="o")
            nc.vector.tensor_copy(o, p)
            nc.sync.dma_start(out=kt_o[:, do, :], in_=o)

        for ks in range(4):
            for h in range(2):
                p = ps.tile([P, KCHUNK], F32)
                for di in range(D_TILES):
                    nc.tensor.matmul(p, xs[:, di, ks * P:(ks + 1) * P],
                                     wv_sb[:, h, di, :],
                                     start=(di == 0), stop=(di == D_TILES - 1))
                o = opool.tile([P, KCHUNK], BF16, tag="o")
                nc.vector.tensor_copy(o, p)
                nc.sync.dma_start(out=v_o[:, ks, h * 512:(h + 1) * 512], in_=o)

        for do in range(D_TILES):
            p = ps.tile([P, RPC], F32)
            for di in range(D_TILES):
                nc.tensor.matmul(p, wq_sb[:, do, di, :],
                                 xq_sb[:, di, :],
                                 start=(di == 0), stop=(di == D_TILES - 1))
            o = opool.tile([P, RPC], BF16, tag="o")
            nc.vector.tensor_copy(o, p)
            nc.sync.dma_start(out=qt_o[:, do, :], in_=o)
    nc.compile()
    return nc


# ---------------------------------------------------------------- NEFF 2
def _build_nc2():
    nc = bacc.Bacc("TRN2", target_bir_lowering=False, debug=False,
                   num_devices=N_CORES)
    ktf = nc.dram_tensor("ktf", [SEQ_CHUNKS, P, D_TILES, KCHUNK], BF16,
                         kind="ExternalInput").ap()
    vf = nc.dram_tensor("vf", [SEQ_CHUNKS, P, 4, D], BF16,
                        kind="ExternalInput").ap()
    qt = nc.dram_tensor("qt", [P, D_TILES, RPC], BF16,
                        kind="ExternalInput").ap()
    wthr = nc.dram_tensor("wthr", [P, N_QTILES * SEQ_CHUNKS], F32,
                          kind="ExternalInput").ap()
    out = nc.dram_tensor("out", [RPC, D], F32, kind="ExternalOutput").ap()
    out_t = out.rearrange("(t p) f -> p t f", p=P)

    with tile.TileContext(nc) as tc, ExitStack() as ctx:
        _attention(ctx, tc, ktf, vf, qt, wthr, out_t)
    nc.compile()
    return nc


def _attention(ctx, tc, ktf, vf, qt_in, wthr, out_t):
    nc = tc.nc
    AX = mybir.AxisListType
    OP = mybir.AluOpType
    ACT = mybir.ActivationFunctionType

    consts = ctx.enter_context(tc.tile_pool(name="consts", bufs=1))
    qt_pool = ctx.enter_context(tc.tile_pool(name="qt", bufs=1))
    kt_pool = ctx.enter_context(tc.tile_pool(name="kt", bufs=3))
    vv_pool = ctx.enter_context(tc.tile_pool(name="vv", bufs=3))
    p_pool = ctx.enter_context(tc.tile_pool(name="p", bufs=3))
    pt_pool = ctx.enter_context(tc.tile_pool(name="pt", bufs=3))
    o_pool = ctx.enter_context(tc.tile_pool(name="obig", bufs=1))
    osb_pool = ctx.enter_context(tc.tile_pool(name="osb", bufs=2))
    stat_pool = ctx.enter_context(tc.tile_pool(name="stat", bufs=48))

    s_ps = ctx.enter_context(tc.tile_pool(name="s_ps", bufs=2, space="PSUM"))
    t_ps = ctx.enter_context(tc.tile_pool(name="t_ps", bufs=2, space="PSUM"))
    o_ps = ctx.enter_context(tc.tile_pool(name="o_ps", bufs=2, space="PSUM"))

    qt_sb = qt_pool.tile([P, D_TILES, RPC], BF16)
    nc.sync.dma_start(out=qt_sb[:], in_=qt_in)

    ident = consts.tile([P, P], BF16)
    make_identity(nc, ident)
    iota_i = consts.tile([P, KCHUNK], mybir.dt.int32)
    nc.gpsimd.iota(iota_i, pattern=[[1, KCHUNK]], base=0, channel_multiplier=0)
    iota_f = consts.tile([P, KCHUNK], F32)
    nc.vector.tensor_copy(iota_f, iota_i)
    wthr_sb = consts.tile([P, N_QTILES * SEQ_CHUNKS], F32)
    nc.sync.dma_start(out=wthr_sb[:], in_=wthr)

    negbig = consts.tile([P, KCHUNK], F32)
    nc.gpsimd.memset(negbig, NEG_BIG)
    masks = consts.tile([P, N_PAIRS, KCHUNK], F32)
    pair_idx = {}
    for t in range(N_QTILES):
        for j in range(TILE_CHUNKS[t]):
            pair_idx[(t, j)] = len(pair_idx)

    o_all = o_pool.tile([P, N_QTILES, D], F32)
    m_run = [None] * N_QTILES
    l_run = [None] * N_QTILES

    for j in range(SEQ_CHUNKS):
        ktj = kt_pool.tile([P, D_TILES, KCHUNK], BF16, tag="kt")
        nc.sync.dma_start(out=ktj[:], in_=ktf[j])
        vj = vv_pool.tile([P, 4, D], BF16)
        nc.sync.dma_start(out=vj[:], in_=vf[j])

        for t in range(N_QTILES):
            if j >= TILE_CHUNKS[t]:
                continue
            first = (j == 0)

            sps = s_ps.tile([P, KCHUNK], F32)
            for do in range(D_TILES):
                nc.tensor.matmul(sps, qt_sb[:, do, t * P:(t + 1) * P],
                                 ktj[:, do, :],
                                 start=(do == 0), stop=(do == D_TILES - 1))

            # build this pair's additive causal mask at first use
            col = t * SEQ_CHUNKS + j
            m_sl = masks[:, pair_idx[(t, j)], :]
            nc.vector.scalar_tensor_tensor(m_sl, iota_f,
                                           wthr_sb[:, col:col + 1], negbig,
                                           op0=OP.is_ge, op1=OP.mult)
            nc.vector.tensor_tensor(sps, sps, m_sl, OP.add)
            rmax_s = stat_pool.tile([P, 1], F32, tag="stat")
            nc.vector.reduce_max(rmax_s, sps, axis=AX.X)

            # nm = -running_max of scaled scores (exp bias)
            nm = stat_pool.tile([P, 1], F32, tag="stat")
            if first:
                nc.vector.tensor_scalar_mul(nm, rmax_s, -SM_SCALE)
            else:
                nc.vector.scalar_tensor_tensor(nm, rmax_s, -SM_SCALE, m_run[t],
                                               op0=OP.mult, op1=OP.min)

            p_sb = p_pool.tile([P, KCHUNK], BF16)
            rsum = stat_pool.tile([P, 1], F32, tag="stat")
            nc.scalar.activation(p_sb, sps, ACT.Exp, bias=nm, scale=SM_SCALE,
                                 accum_out=rsum)

            if first:
                l_new = rsum
            else:
                # alpha = exp(nm_new - nm_old)
                alpha = stat_pool.tile([P, 1], F32, tag="stat")
                nc.scalar.activation(alpha, m_run[t], ACT.Exp, bias=nm,
                                     scale=-1.0)
                l_new = stat_pool.tile([P, 1], F32, tag="stat")
                nc.vector.scalar_tensor_tensor(l_new, l_run[t], alpha, rsum,
                                               op0=OP.mult, op1=OP.add)
            m_run[t], l_run[t] = nm, l_new

            tps = t_ps.tile([P, KCHUNK], BF16)
            for ks in range(4):
                nc.tensor.transpose(tps[:, ks * P:(ks + 1) * P],
                                    p_sb[:, ks * P:(ks + 1) * P], ident)
            ptj = pt_pool.tile([P, KCHUNK], BF16)
            nc.vector.tensor_copy(ptj, tps)

            ops = o_ps.tile([P, D], F32)
            for h in range(2):
                for ks in range(4):
                    nc.tensor.matmul(ops[:, h * 512:(h + 1) * 512],
                                     ptj[:, ks * P:(ks + 1) * P],
                                     vj[:, ks, h * 512:(h + 1) * 512],
                                     start=(ks == 0), stop=(ks == 3))
            o_slice = o_all[:, t, :]
            if first:
                nc.vector.tensor_copy(o_slice, ops)
            else:
                nc.vector.scalar_tensor_tensor(o_slice, o_slice, alpha, ops,
                                               op0=OP.mult, op1=OP.add)

    for t in range(N_QTILES):
        recip = stat_pool.tile([P, 1], F32, tag="stat")
        nc.vector.reciprocal(recip, l_run[t])
        osb = osb_pool.tile([P, D], F32)
        nc.vector.tensor_scalar_mul(osb, o_all[:, t, :], recip)
        nc.sync.dma_start(out=out_t[:, t, :], in_=osb)


def _get_ncs():
    if "nc1" not in _CACHE:
        _CACHE["nc1"] = _build_nc1()
        _CACHE["nc2"] = _build_nc2()
    return _CACHE["nc1"], _CACHE["nc2"]


def _qcols(c):
    blocks = [8 * t + c for t in range(N_QTILES)]
    return blocks, np.concatenate(
        [np.arange(b * P, (b + 1) * P) for b in blocks])


def _perm_x(xT_slice):
    """[D, W] -> [128, 8, W] with di_inner on partitions."""
    W = xT_slice.shape[1]
    return np.ascontiguousarray(
        xT_slice.reshape(D_TILES, P, W).transpose(1, 0, 2))


def _perm_w_chunks(wT):
    """[d_in, d_out] -> [8, 128, 8, 128]: [do_chunk, di_p, di_o, do_i]."""
    return np.ascontiguousarray(
        wT.reshape(D_TILES, P, D_TILES, P).transpose(2, 1, 0, 3))


def _perm_w_halves(wT):
    """[d_in, d_out] -> [2, 128, 8, 512]: [half, di_p, di_o, do_i]."""
    return np.ascontiguousarray(
        wT.reshape(D_TILES, P, 2, KCHUNK).transpose(2, 1, 0, 3))


def _phase1_inmaps(xT, wqT, wkT, wvT):
    wk_p = _perm_w_chunks(wkT)
    wq_p = _perm_w_chunks(wqT)
    wv_p = _perm_w_halves(wvT)
    maps = []
    for c in range(N_CORES):
        _, cols = _qcols(c)
        maps.append({
            "xc": _perm_x(xT[:, c * KCHUNK:(c + 1) * KCHUNK]),
            "xq": _perm_x(xT[:, cols]),
            "wq": wq_p, "wk": wk_p, "wv": wv_p})
    return maps


def _phase2_inmaps(ktf, vf, qts):
    maps = []
    r = np.arange(P)
    for c in range(N_CORES):
        blocks, _ = _qcols(c)
        wthr = np.zeros((P, N_QTILES * SEQ_CHUNKS), np.float32)
        for t, B in enumerate(blocks):
            for j in range(TILE_CHUNKS[t]):
                wthr[:, t * SEQ_CHUNKS + j] = np.clip(
                    128 * B + r + 1 - KCHUNK * j, 0, KCHUNK)
        maps.append({"ktf": ktf, "vf": vf, "qt": qts[c], "wthr": wthr})
    return maps


def kernel(x, w_q, w_k, w_v):
    nc1, nc2 = _get_ncs()
    bf = ml_dtypes.bfloat16
    xT = np.ascontiguousarray(x.T).astype(bf)
    wqT = np.ascontiguousarray(w_q.T).astype(bf)
    wkT = np.ascontiguousarray(w_k.T).astype(bf)
    wvT = np.ascontiguousarray(w_v.T).astype(bf)

    res1 = run_bass_kernel_spmd(nc1, _phase1_inmaps(xT, wqT, wkT, wvT),
                                list(range(N_CORES)))
    ktf = np.stack([res1.results[c]["kt"] for c in range(N_CORES)])
    vf = np.stack([res1.results[c]["v"] for c in range(N_CORES)])
    qts = [res1.results[c]["qt"] for c in range(N_CORES)]

    res2 = run_bass_kernel_spmd(nc2, _phase2_inmaps(ktf, vf, qts),
                                list(range(N_CORES)))

    full = np.empty((SEQ, D), np.float32)
    for c in range(N_CORES):
        oc = res2.results[c]["out"]
        blocks, _ = _qcols(c)
        for t, B in enumerate(blocks):
            full[B * P:(B + 1) * P, :] = oc[t * P:(t + 1) * P, :]
    return full


# revision 14
# speedup vs baseline: 1.1255x; 1.0630x over previous
"""Causal attention on 8 TRN2 NeuronCores — two-phase version.

Phase 1 (NEFF-1): Q/K/V projections. K/V sharded over seq across cores;
Q^T computed for the core's own (strided) row blocks.
Host: stack the per-core K^T / V shards (pure data movement).
Phase 2 (NEFF-2): flash-style causal attention, Q rows sharded over cores
(strided 128-row blocks), K^T/V streamed chunk-wise from DRAM.

All DRAM tensors use SBUF-mirroring layouts (partition dim first) so every
DMA is contiguous per partition.
"""

import numpy as np
import ml_dtypes
from contextlib import ExitStack

import concourse.bass as bass
import concourse.tile as tile
from concourse import bacc, mybir
from concourse.bass_utils import run_bass_kernel_spmd
from concourse.masks import make_identity

P = 128
SEQ = 4096
D = 1024
N_CORES = 8
RPC = SEQ // N_CORES          # 512
D_TILES = D // P              # 8
KCHUNK = 512
SEQ_CHUNKS = SEQ // KCHUNK    # 8
N_QTILES = RPC // P           # 4
TILE_CHUNKS = [2, 4, 6, 8]
N_PAIRS = sum(TILE_CHUNKS)    # 20
SM_SCALE = 1.0 / 32.0
NEG_BIG = -1.0e9

BF16 = mybir.dt.bfloat16
F32 = mybir.dt.float32

_CACHE = {}


# ---------------------------------------------------------------- NEFF 1
def _build_nc1():
    nc = bacc.Bacc("TRN2", target_bir_lowering=False, debug=False,
                   num_devices=N_CORES)
    # pre-permuted layouts: partition dim first, contiguous per partition
    xc = nc.dram_tensor("xc", [P, D_TILES, KCHUNK], BF16,
                        kind="ExternalInput").ap()
    xq = nc.dram_tensor("xq", [P, D_TILES, RPC], BF16,
                        kind="ExternalInput").ap()
    wk = nc.dram_tensor("wk", [D_TILES, P, D_TILES, P], BF16,
                        kind="ExternalInput").ap()
    wq = nc.dram_tensor("wq", [D_TILES, P, D_TILES, P], BF16,
                        kind="ExternalInput").ap()
    wv = nc.dram_tensor("wv", [2, P, D_TILES, KCHUNK], BF16,
                        kind="ExternalInput").ap()
    kt_o = nc.dram_tensor("kt", [P, D_TILES, KCHUNK], BF16,
                          kind="ExternalOutput").ap()
    v_o = nc.dram_tensor("v", [P, 4, D], BF16, kind="ExternalOutput").ap()
    qt_o = nc.dram_tensor("qt", [P, D_TILES, RPC], BF16,
                          kind="ExternalOutput").ap()

    with tile.TileContext(nc) as tc, ExitStack() as ctx:
        wpool = ctx.enter_context(tc.tile_pool(name="w", bufs=1))
        xpool = ctx.enter_context(tc.tile_pool(name="x", bufs=1))
        opool = ctx.enter_context(tc.tile_pool(name="o", bufs=6))
        ps = ctx.enter_context(tc.tile_pool(name="ps", bufs=4, space="PSUM"))

        xs = xpool.tile([P, D_TILES, KCHUNK], BF16, tag="xs")
        for di in range(D_TILES):
            nc.sync.dma_start(out=xs[:, di, :], in_=xc[:, di, :])

        # weight SBUF layouts mirror the chunked DRAM layouts:
        # wk_sb/wq_sb: [di_p, do_chunk, di_o, do_i]; wv_sb: [di_p, half, di_o, do_i]
        wk_sb = wpool.tile([P, D_TILES, D_TILES, P], BF16, tag="wk")
        wq_sb = wpool.tile([P, D_TILES, D_TILES, P], BF16, tag="wq")
        wv_sb = wpool.tile([P, 2, D_TILES, KCHUNK], BF16, tag="wv")
        for do in range(D_TILES):
            nc.sync.dma_start(out=wk_sb[:, do], in_=wk[do])
        for h in range(2):
            nc.sync.dma_start(out=wv_sb[:, h], in_=wv[h])
        xq_sb = xpool.tile([P, D_TILES, RPC], BF16, tag="xq")
        nc.sync.dma_start(out=xq_sb[:], in_=xq)
        for do in range(D_TILES):
            nc.sync.dma_start(out=wq_sb[:, do], in_=wq[do])

        for do in range(D_TILES):
            p = ps.tile([P, KCHUNK], F32)
            for di in range(D_TILES):
                nc.tensor.matmul(p, wk_sb[:, do, di, :],
                                 xs[:, di, :],
                                 start=(di == 0), stop=(di == D_TILES - 1))
            o = opool.tile([P, KCHUNK], BF16, tag="o")
            nc.vector.tensor_copy(o, p)
            nc.sync.dma_start(out=kt_o[:, do, :], in_=o)

        for ks in range(4):
            for h in range(2):
                p = ps.tile([P, KCHUNK], F32)
                for di in range(D_TILES):
                    nc.tensor.matmul(p, xs[:, di, ks * P:(ks + 1) * P],
                                     wv_sb[:, h, di, :],
                                     start=(di == 0), stop=(di == D_TILES - 1))
                o = opool.tile([P, KCHUNK], BF16, tag="o")
                nc.vector.tensor_copy(o, p)
                nc.sync.dma_start(out=v_o[:, ks, h * 512:(h + 1) * 512], in_=o)

        for do in range(D_TILES):
            p = ps.tile([P, RPC], F32)
            for di in range(D_TILES):
                nc.tensor.matmul(p, wq_sb[:, do, di, :],
                                 xq_sb[:, di, :],
                                 start=(di == 0), stop=(di == D_TILES - 1))
            o = opool.tile([P, RPC], BF16, tag="o")
            nc.vector.tensor_copy(o, p)
            nc.sync.dma_start(out=qt_o[:, do, :], in_=o)
    nc.compile()
    return nc


# ---------------------------------------------------------------- NEFF 2
def _build_nc2():
    nc = bacc.Bacc("TRN2", target_bir_lowering=False, debug=False,
                   num_devices=N_CORES)
    ktf = nc.dram_tensor("ktf", [SEQ_CHUNKS, P, D_TILES, KCHUNK], BF16,
                         kind="ExternalInput").ap()
    vf = nc.dram_tensor("vf", [SEQ_CHUNKS, P, 4, D], BF16,
                        kind="ExternalInput").ap()
    qt = nc.dram_tensor("qt", [P, D_TILES, RPC], BF16,
                        kind="ExternalInput").ap()
    wthr = nc.dram_tensor("wthr", [P, N_QTILES * SEQ_CHUNKS], F32,
                          kind="ExternalInput").ap()
    out = nc.dram_tensor("out", [RPC, D], F32, kind="ExternalOutput").ap()
    out_t = out.rearrange("(t p) f -> p t f", p=P)

    with tile.TileContext(nc) as tc, ExitStack() as ctx:
        _attention(ctx, tc, ktf, vf, qt, wthr, out_t)
    nc.compile()
    return nc


def _attention(ctx, tc, ktf, vf, qt_in, wthr, out_t):
    """Two-pass softmax: pass A fills per-tile masked score rows in SBUF
    (K^T streamed, V parked resident); pass B does one max/exp/transpose/AV
    chain per Q tile with the AV accumulation held in PSUM."""
    nc = tc.nc
    AX = mybir.AxisListType
    OP = mybir.AluOpType
    ACT = mybir.ActivationFunctionType

    consts = ctx.enter_context(tc.tile_pool(name="consts", bufs=1))
    qt_pool = ctx.enter_context(tc.tile_pool(name="qt", bufs=1))
    kt_pool = ctx.enter_context(tc.tile_pool(name="kt", bufs=3))
    vres_pool = ctx.enter_context(tc.tile_pool(name="vres", bufs=1))
    srow_pool = ctx.enter_context(tc.tile_pool(name="srow", bufs=1))
    mask_pool = ctx.enter_context(tc.tile_pool(name="mask", bufs=3))
    p_pool = ctx.enter_context(tc.tile_pool(name="p", bufs=2))
    pt_pool = ctx.enter_context(tc.tile_pool(name="pt", bufs=2))
    osb_pool = ctx.enter_context(tc.tile_pool(name="osb", bufs=2))
    stat_pool = ctx.enter_context(tc.tile_pool(name="stat", bufs=16))

    s_ps = ctx.enter_context(tc.tile_pool(name="s_ps", bufs=2, space="PSUM"))
    t_ps = ctx.enter_context(tc.tile_pool(name="t_ps", bufs=2, space="PSUM"))
    o_ps = ctx.enter_context(tc.tile_pool(name="o_ps", bufs=2, space="PSUM"))

    qt_sb = qt_pool.tile([P, D_TILES, RPC], BF16)
    nc.sync.dma_start(out=qt_sb[:], in_=qt_in)

    ident = consts.tile([P, P], BF16)
    make_identity(nc, ident)
    iota_i = consts.tile([P, KCHUNK], mybir.dt.int32)
    nc.gpsimd.iota(iota_i, pattern=[[1, KCHUNK]], base=0, channel_multiplier=0)
    iota_f = consts.tile([P, KCHUNK], F32)
    nc.vector.tensor_copy(iota_f, iota_i)
    wthr_sb = consts.tile([P, N_QTILES * SEQ_CHUNKS], F32)
    nc.sync.dma_start(out=wthr_sb[:], in_=wthr)
    negbig = consts.tile([P, KCHUNK], F32)
    nc.gpsimd.memset(negbig, NEG_BIG)

    # per-tile score rows (exact-size slots via distinct tags)
    s_rows = [srow_pool.tile([P, TILE_CHUNKS[t], KCHUNK], F32, tag=f"s{t}",
                             name=f"srow{t}")
              for t in range(N_QTILES)]
    v_res = [None] * SEQ_CHUNKS

    # ---- pass A: stream K^T, park V, fill masked score rows -------------
    for j in range(SEQ_CHUNKS):
        ktj = kt_pool.tile([P, D_TILES, KCHUNK], BF16, tag="kt")
        nc.sync.dma_start(out=ktj[:], in_=ktf[j])
        v_res[j] = vres_pool.tile([P, 4, D], BF16, tag=f"v{j}", name=f"vres{j}")
        nc.sync.dma_start(out=v_res[j][:], in_=vf[j])

        for t in range(N_QTILES):
            if j >= TILE_CHUNKS[t]:
                continue
            sps = s_ps.tile([P, KCHUNK], F32)
            for do in range(D_TILES):
                nc.tensor.matmul(sps, qt_sb[:, do, t * P:(t + 1) * P],
                                 ktj[:, do, :],
                                 start=(do == 0), stop=(do == D_TILES - 1))
            col = t * SEQ_CHUNKS + j
            m_sl = mask_pool.tile([P, KCHUNK], F32, tag="mask")
            nc.vector.scalar_tensor_tensor(m_sl, iota_f,
                                           wthr_sb[:, col:col + 1], negbig,
                                           op0=OP.is_ge, op1=OP.mult)
            nc.vector.tensor_tensor(s_rows[t][:, j, :], sps, m_sl, OP.add)

    # ---- pass B: per-tile softmax + P^T + AV ----------------------------
    for t in range(N_QTILES):
        n = TILE_CHUNKS[t]
        srow = s_rows[t]

        rmax = stat_pool.tile([P, 1], F32, tag="stat")
        nc.vector.reduce_max(rmax, srow, axis=AX.XY)
        nm = stat_pool.tile([P, 1], F32, tag="stat")
        nc.vector.tensor_scalar_mul(nm, rmax, -SM_SCALE)

        p_sb = p_pool.tile([P, SEQ_CHUNKS, KCHUNK], BF16, tag="p")
        rsum = stat_pool.tile([P, 1], F32, tag="stat")
        nc.scalar.activation(p_sb[:, :n, :], srow, ACT.Exp, bias=nm,
                             scale=SM_SCALE, accum_out=rsum)
        recip = stat_pool.tile([P, 1], F32, tag="stat")
        nc.vector.reciprocal(recip, rsum)

        ptj = pt_pool.tile([P, SEQ_CHUNKS, KCHUNK], BF16, tag="pt")
        for kc in range(n):
            tps = t_ps.tile([P, KCHUNK], BF16)
            for ks in range(4):
                nc.tensor.transpose(tps[:, ks * P:(ks + 1) * P],
                                    p_sb[:, kc, ks * P:(ks + 1) * P], ident)
            nc.scalar.copy(ptj[:, kc, :], tps)

        ops = o_ps.tile([P, D], F32)
        for h in range(2):
            for kc in range(n):
                for ks in range(4):
                    nc.tensor.matmul(
                        ops[:, h * 512:(h + 1) * 512],
                        ptj[:, kc, ks * P:(ks + 1) * P],
                        v_res[kc][:, ks, h * 512:(h + 1) * 512],
                        start=(kc == 0 and ks == 0),
                        stop=(kc == n - 1 and ks == 3))
        osb = osb_pool.tile([P, D], F32)
        nc.vector.tensor_scalar_mul(osb, ops, recip)
        nc.sync.dma_start(out=out_t[:, t, :], in_=osb)


def _get_ncs():
    if "nc1" not in _CACHE:
        _CACHE["nc1"] = _build_nc1()
        _CACHE["nc2"] = _build_nc2()
    return _CACHE["nc1"], _CACHE["nc2"]


def _qcols(c):
    blocks = [8 * t + c for t in range(N_QTILES)]
    return blocks, np.concatenate(
        [np.arange(b * P, (b + 1) * P) for b in blocks])


def _perm_x(xT_slice):
    """[D, W] -> [128, 8, W] with di_inner on partitions."""
    W = xT_slice.shape[1]
    return np.ascontiguousarray(
        xT_slice.reshape(D_TILES, P, W).transpose(1, 0, 2))


def _perm_w_chunks(wT):
    """[d_in, d_out] -> [8, 128, 8, 128]: [do_chunk, di_p, di_o, do_i]."""
    return np.ascontiguousarray(
        wT.reshape(D_TILES, P, D_TILES, P).transpose(2, 1, 0, 3))


def _perm_w_halves(wT):
    """[d_in, d_out] -> [2, 128, 8, 512]: [half, di_p, di_o, do_i]."""
    return np.ascontiguousarray(
        wT.reshape(D_TILES, P, 2, KCHUNK).transpose(2, 1, 0, 3))


def _phase1_inmaps(xT, wqT, wkT, wvT):
    wk_p = _perm_w_chunks(wkT)
    wq_p = _perm_w_chunks(wqT)
    wv_p = _perm_w_halves(wvT)
    maps = []
    for c in range(N_CORES):
        _, cols = _qcols(c)
        maps.append({
            "xc": _perm_x(xT[:, c * KCHUNK:(c + 1) * KCHUNK]),
            "xq": _perm_x(xT[:, cols]),
            "wq": wq_p, "wk": wk_p, "wv": wv_p})
    return maps


def _phase2_inmaps(ktf, vf, qts):
    maps = []
    r = np.arange(P)
    for c in range(N_CORES):
        blocks, _ = _qcols(c)
        wthr = np.zeros((P, N_QTILES * SEQ_CHUNKS), np.float32)
        for t, B in enumerate(blocks):
            for j in range(TILE_CHUNKS[t]):
                wthr[:, t * SEQ_CHUNKS + j] = np.clip(
                    128 * B + r + 1 - KCHUNK * j, 0, KCHUNK)
        maps.append({"ktf": ktf, "vf": vf, "qt": qts[c], "wthr": wthr})
    return maps


def _run_spmd(nc, in_maps):
    """run_bass_kernel_spmd with retries: the first device touch after a
    crashed process occasionally reports NRT_EXEC_UNIT_UNRECOVERABLE once."""
    last = None
    for _ in range(3):
        try:
            return run_bass_kernel_spmd(nc, in_maps, list(range(N_CORES)))
        except Exception as e:  # transient device wedge
            last = e
    raise last


def kernel(x, w_q, w_k, w_v):
    nc1, nc2 = _get_ncs()
    bf = ml_dtypes.bfloat16
    x = np.asarray(x)
    xT = np.ascontiguousarray(x.T).astype(bf)
    wqT = np.ascontiguousarray(np.asarray(w_q).T).astype(bf)
    wkT = np.ascontiguousarray(np.asarray(w_k).T).astype(bf)
    wvT = np.ascontiguousarray(np.asarray(w_v).T).astype(bf)

    res1 = _run_spmd(nc1, _phase1_inmaps(xT, wqT, wkT, wvT))
    ktf = np.stack([res1.results[c]["kt"] for c in range(N_CORES)])
    vf = np.stack([res1.results[c]["v"] for c in range(N_CORES)])
    qts = [res1.results[c]["qt"] for c in range(N_CORES)]

    res2 = _run_spmd(nc2, _phase2_inmaps(ktf, vf, qts))

    full = np.empty((SEQ, D), np.float32)
    for c in range(N_CORES):
        oc = res2.results[c]["out"]
        blocks, _ = _qcols(c)
        for t, B in enumerate(blocks):
            full[B * P:(B + 1) * P, :] = oc[t * P:(t + 1) * P, :]
    return full
